# revision 2
# baseline (speedup 1.0000x reference)
# Trainium2 Bass kernel for nn_DirectRanker (ragged_sequence).
#
# Math shortcut: result = tanh((sorted_enc[:,1:,:] - sorted_enc[:,:1,:]) @ W.T)
# commutes with the linear map, so per-row scores s = encodes @ W.T are
# computed FIRST (the memory-bound part: 512 MiB of fp16 streamed once), and
# the per-group sort/diff/tanh runs on the tiny [N] score vector:
#   result[g, k-1] = tanh(s_sorted[g, k] - s_sorted[g, 0]),  k = 1..63
#
# Sharding: groups split across 8 cores (2048 groups/core), no cross-core
# communication.
#
# v2 layout: encodes is transposed ON HOST to [2, 128, ROWS] fp16 (d-chunk,
# d, row), so TensorE computes the matvec in its NATIVE orientation:
#   psum[1, 512] += W_chunk[128, 1].T @ ET_chunk[128, 512]
# (2 cycles/row on PE, no on-chip transposes).  ScalarE exits the [1, 2048]
# psum spans to SBUF fp16, and a single SBUF->SBUF DMA per 128-group tile
# relayouts the flat score vector into [group(partition), elem(free)] for
# the sort.  DVE only sorts; gpsimd scatters; Act does keys/exits/tanh.
#
# Exact stable argsort over y within each 64-row group: integer keys
#   key = (y * 2^23 + 2^23) * 64 | elem_index     (y is a multiple of 2^-23)
# are sorted through their f32 bitcast views (monotone for positive int32;
# keys lie in [2^29, 2^30) so the views are normal floats) with 8 rounds of
# DVE max8 + match_replace; perm = low 6 bits of the sorted keys. The score
# permutation runs on gpsimd local_scatter (fp16 as int16), ranks coming
# from scattering a descending iota by perm.
import os
from contextlib import ExitStack

import numpy as np

import concourse.bacc as bacc
import concourse.mybir as mybir
import concourse.tile as tile
from concourse.bass_utils import run_bass_kernel_spmd

N_CORES = 8
N = 1048576
D = 256
G = 64
NG = N // G                # 16384 groups
ROWS = N // N_CORES        # 131072 rows per core
GPC = NG // N_CORES        # 2048 groups per core
T_TILES = GPC // 128       # 16 tiles of 128 groups (8192 rows) per core
RPT = 128 * G              # rows per tile = 8192
MM_N = 512                 # moving free size per matmul (1 psum bank)
EXIT_N = 2048              # scores per Act exit copy (4 psum banks)
F32 = mybir.dt.float32
F16 = mybir.dt.float16
I32 = mybir.dt.int32
I16 = mybir.dt.int16
Alu = mybir.AluOpType
Act = mybir.ActivationFunctionType

_built = {}


def _build_nc():
    nc = bacc.Bacc("TRN2", target_bir_lowering=False, debug=False,
                   num_devices=N_CORES)
    # host-transposed encodes: [d-chunk, d-in-chunk, row]
    et_in = nc.dram_tensor("et", [2, 128, ROWS], F16, kind="ExternalInput")
    y_in = nc.dram_tensor("y_coord", [ROWS], F32, kind="ExternalInput")
    w_in = nc.dram_tensor("w", [1, D], F32, kind="ExternalInput")
    out = nc.dram_tensor("result", [GPC * (G - 1)], F32, kind="ExternalOutput")

    y_r = y_in.ap().rearrange("(t p u) -> t p u", p=128, u=G)
    out_r = out.ap().rearrange("(t p k) -> t p k", p=128, k=G - 1)

    with tile.TileContext(nc) as tc, ExitStack() as ctx:
        const_pool = ctx.enter_context(tc.tile_pool(name="const", bufs=1))
        epool = ctx.enter_context(tc.tile_pool(name="e", bufs=2))
        sfpool = ctx.enter_context(tc.tile_pool(name="sf", bufs=2))
        spool = ctx.enter_context(tc.tile_pool(name="s", bufs=3))
        scr_pool = ctx.enter_context(tc.tile_pool(name="scr", bufs=3))
        ps_pool = ctx.enter_context(
            tc.tile_pool(name="ps", bufs=2, space="PSUM"))

        # W with d on partitions: wsb[:, c] = W[c*128:(c+1)*128]
        wsb = const_pool.tile([128, 2], F32)
        nc.sync.dma_start(wsb[:],
                          w_in.ap()[0, :].rearrange("(c p) -> p c", p=128))
        wsb_h = const_pool.tile([128, 2], F16)
        nc.vector.tensor_copy(wsb_h[:], wsb[:])
        # free-dim iota (elem index within group) for the sort keys
        iota_i = const_pool.tile([128, G], I32)
        nc.gpsimd.iota(iota_i[:], pattern=[[1, G]], base=0, channel_multiplier=0)
        # descending iota (63..0) as int16: data for the rank-producing scatter
        iota_d16 = const_pool.tile([128, G], I16)
        nc.gpsimd.iota(iota_d16[:], pattern=[[-1, G]], base=G - 1,
                       channel_multiplier=0)

        for T in range(T_TILES):
            # ---- stream the transposed encodes for this tile's 8192 rows ----
            et0 = epool.tile([128, RPT], F16, tag="et0")
            nc.sync.dma_start(et0[:], et_in.ap()[0, :, T * RPT:(T + 1) * RPT])
            et1 = epool.tile([128, RPT], F16, tag="et1")
            nc.sync.dma_start(et1[:], et_in.ap()[1, :, T * RPT:(T + 1) * RPT])

            # ---- scores: psum[1, n] = dot(W, E[row n]) via native matvec ----
            sflat = sfpool.tile([1, RPT], F16, tag="sflat")
            for j in range(RPT // EXIT_N):
                ps = ps_pool.tile([1, EXIT_N], F32, tag="ps")
                for q in range(EXIT_N // MM_N):
                    c0 = j * EXIT_N + q * MM_N
                    nc.tensor.matmul(ps[:, q * MM_N:(q + 1) * MM_N],
                                     wsb_h[:, 0:1], et0[:, c0:c0 + MM_N],
                                     start=True, stop=False)
                    nc.tensor.matmul(ps[:, q * MM_N:(q + 1) * MM_N],
                                     wsb_h[:, 1:2], et1[:, c0:c0 + MM_N],
                                     start=False, stop=True)
                # exit psum -> sbuf fp16 (Act)
                nc.scalar.copy(sflat[:, j * EXIT_N:(j + 1) * EXIT_N], ps[:])

            # ---- relayout flat scores -> [group(partition), elem] ----
            s_t = spool.tile([128, G], F16, tag="s")
            nc.sync.dma_start(s_t[:], sflat[:])

            # ---- keys from y ----
            y_t = spool.tile([128, G], F32, tag="y")
            nc.sync.dma_start(y_t[:], y_r[T])
            ki = spool.tile([128, G], I32, tag="ki")
            nc.scalar.activation(ki[:], y_t[:], Act.Copy,
                                 bias=float(1 << 23), scale=float(1 << 23))
            k64 = spool.tile([128, G], I32, tag="k64")
            nc.scalar.activation(k64[:], ki[:], Act.Copy, bias=0.0, scale=64.0)
            keys = spool.tile([128, G], I32, tag="keys")
            nc.vector.tensor_tensor(out=keys[:], in0=k64[:], in1=iota_i[:],
                                    op=Alu.bitwise_or)

            # ---- full descending sort of the int keys on DVE via 8 rounds of
            # max8 + match_replace (compares run on the f32 bitcast views,
            # which order identically to the positive int32 keys) ----
            sorted_i = spool.tile([128, G], I32, tag="sorted")
            wka = scr_pool.tile([128, G], I32, tag="wka")
            wkb = scr_pool.tile([128, G], I32, tag="wkb")
            src = keys
            dst = wka
            for r in range(8):
                nc.vector.max(sorted_i[:, r * 8:(r + 1) * 8].bitcast(F32),
                              src[:].bitcast(F32))
                if r < 7:
                    nc.vector.match_replace(
                        dst[:].bitcast(F32),
                        sorted_i[:, r * 8:(r + 1) * 8].bitcast(F32),
                        src[:].bitcast(F32), 0.0)
                    src, dst = dst, (wkb if dst is wka else wka)

            # perm (descending argsort) = low 6 bits of the sorted keys
            perm32 = scr_pool.tile([128, G], I32, tag="perm32")
            nc.vector.tensor_scalar(out=perm32[:], in0=sorted_i[:], scalar1=63,
                                    scalar2=None, op0=Alu.bitwise_and)
            perm16 = spool.tile([128, G], I16, tag="perm16")
            nc.scalar.copy(perm16[:], perm32[:])
            # rank_asc[i] = position of element i in ascending order:
            # scatter descending iota by perm
            rank16 = spool.tile([128, G], I16, tag="rank16")
            nc.gpsimd.local_scatter(rank16[:], iota_d16[:], perm16[:],
                                    channels=128, num_elems=G, num_idxs=G)

            # ---- permute fp16 scores by rank in one gpsimd scatter ----
            ssort = spool.tile([128, G], I16, tag="ssort")
            nc.gpsimd.local_scatter(ssort[:], s_t[:].bitcast(I16), rank16[:],
                                    channels=128, num_elems=G, num_idxs=G)
            ssf = ssort[:].bitcast(F16)

            # ---- result tile: tanh(ssort[:, 1:] - ssort[:, 0]) ----
            negs0 = spool.tile([128, 1], F32, tag="negs0")
            nc.scalar.mul(negs0[:], ssf[:, 0:1], -1.0)
            th = spool.tile([128, G - 1], F32, tag="th")
            nc.scalar.activation(th[:], ssf[:, 1:G], Act.Tanh,
                                 bias=negs0[:], scale=1.0)
            nc.sync.dma_start(out_r[T], th[:])

    nc.compile()
    return nc


last_results = None


def kernel(encodes, y_coord, W, x_coord=None):
    global last_results
    if "nc" not in _built:
        _built["nc"] = _build_nc()
    nc = _built["nc"]

    e16 = np.asarray(encodes, dtype=np.float16)
    y_coord = np.ascontiguousarray(np.asarray(y_coord, dtype=np.float32))
    W = np.ascontiguousarray(np.asarray(W, dtype=np.float32))

    in_maps = []
    for c in range(N_CORES):
        et_c = np.ascontiguousarray(e16[c * ROWS:(c + 1) * ROWS].T)
        in_maps.append({
            "et": et_c.reshape(2, 128, ROWS),
            "y_coord": y_coord[c * ROWS:(c + 1) * ROWS],
            "w": W,
        })
    # Only request tracing when the axon NTFF hook is importable; otherwise
    # force it off (bass_utils would crash importing antenv.axon_hooks if
    # BASS_TRACE leaked into the environment without the shim installed).
    want_trace = bool(os.environ.get("BASS_TRACE"))
    if want_trace:
        try:
            from antenv.axon_hooks import get_axon_ntff_profile_hook  # noqa: F401
        except ImportError:
            want_trace = False
            os.environ["BASS_NEVER_TRACE"] = "1"
    res = run_bass_kernel_spmd(
        nc, in_maps, core_ids=list(range(N_CORES)),
        trace=want_trace,
    )
    last_results = res
    result = np.concatenate([r["result"] for r in res.results])
    polarity = np.ones(NG * (G - 1), dtype=np.float32)
    return result, polarity
